# revision 1
# baseline (speedup 1.0000x reference)
"""BiLSTM encoder (nn_BiLstmCellEncoder) as a Bass/TRN2 SPMD kernel.

Full inputs in, full output out. Sharding: direction x batch over 8
NeuronCores — cores 0-3 run the forward LSTM, cores 4-7 the backward
LSTM (on host-time-reversed x), each on a batch shard of 16. The
backward cores un-flip their hidden-state stores with a data-driven
indirect DMA, one pairwise AllReduce merges the attention
pre-activations, and each core emits its direction's half of the
pooled output; the host assembles the [64, 1024] result.

Problem sizes are hardcoded: B=64, S=512, D=256, H=512, A=64.
"""

import numpy as np

from concourse import bacc, bass, mybir, tile
from concourse.bass_utils import run_bass_kernel_spmd

F32 = mybir.dt.float32
F32R = mybir.dt.float32r
I32 = mybir.dt.int32
AF = mybir.ActivationFunctionType
ALU = mybir.AluOpType

B, S, D, H, A = 64, 512, 256, 512, 64
N_CORES = 8
B_LOCAL = 16


def build_program(S=S, n_cores=N_CORES, B_local=B_LOCAL, D=D, H=H, A=A):
    G = 4 * H
    SB = S * B_local
    KD = D // 128
    KH = H // 128
    NT = G // 512
    MT = SB // 128
    n_half = n_cores // 2
    pairs = [[i, i + n_half] for i in range(n_half)]

    nc = bacc.Bacc("TRN2", target_bir_lowering=False, debug=False,
                   num_devices=n_cores)

    xT = nc.dram_tensor("xT", [D, SB], F32R, kind="ExternalInput")
    wihT = nc.dram_tensor("wihT", [D, G], F32R, kind="ExternalInput")
    whhT = nc.dram_tensor("whhT", [H, G], F32R, kind="ExternalInput")
    bias = nc.dram_tensor("bias", [1, G], F32, kind="ExternalInput")
    attWT = nc.dram_tensor("attWT", [H, A], F32R, kind="ExternalInput")
    attv = nc.dram_tensor("attv", [A, 1], F32R, kind="ExternalInput")
    eye = nc.dram_tensor("eye", [128, 128], F32, kind="ExternalInput")
    offsT = nc.dram_tensor("offsT", [B_local, S], I32, kind="ExternalInput")
    pooledT = nc.dram_tensor("pooledT", [H, B_local], F32,
                             kind="ExternalOutput")

    ginp = nc.dram_tensor("ginp", [SB, G], F32)
    hstore = nc.dram_tensor("hstore", [SB, H], F32)
    hTstore = nc.dram_tensor("hTstore", [H, SB], F32R)

    with tile.TileContext(nc) as tc:
        with tc.tile_pool(name="persist", bufs=1) as persist:
            eye_sb = persist.tile([128, 128], F32)
            nc.sync.dma_start(out=eye_sb[:], in_=eye[:])
            offs_sb = persist.tile([B_local, S], I32)
            nc.sync.dma_start(out=offs_sb[:], in_=offsT[:])
            whh_sb = persist.tile([128, KH, G], F32R)
            nc.sync.dma_start(
                out=whh_sb[:],
                in_=whhT[:].rearrange("(k p) g -> p k g", p=128))

            # ---- P1: ginp[(s,b), G] = x @ Wih.T + bias ----
            with tc.tile_pool(name="p1", bufs=3) as p1, \
                 tc.tile_pool(name="p1w", bufs=1) as p1w, \
                 tc.tile_pool(name="ps1", bufs=4, space="PSUM") as ps1:
                wih_sb = p1w.tile([128, KD, G], F32R)
                nc.sync.dma_start(
                    out=wih_sb[:],
                    in_=wihT[:].rearrange("(k p) g -> p k g", p=128))
                bias_bc = p1w.tile([128, G], F32)
                nc.gpsimd.dma_start(
                    out=bias_bc[:], in_=bias[0:1, :].broadcast_to((128, G)))
                for j in range(MT):
                    xt = p1.tile([128, KD, 128], F32R)
                    nc.sync.dma_start(
                        out=xt[:],
                        in_=xT[:, 128 * j:128 * (j + 1)].rearrange(
                            "(k p) m -> p k m", p=128))
                    gout = p1.tile([128, G], F32)
                    for n in range(NT):
                        pg = ps1.tile([128, 512], F32)
                        for k in range(KD):
                            nc.tensor.matmul(
                                pg[:], xt[:, k, :],
                                wih_sb[:, k, 512 * n:512 * (n + 1)],
                                start=(k == 0), stop=(k == KD - 1))
                        nc.vector.tensor_add(
                            gout[:, 512 * n:512 * (n + 1)], pg[:],
                            bias_bc[:, 512 * n:512 * (n + 1)])
                    nc.sync.dma_start(
                        out=ginp[128 * j:128 * (j + 1), :], in_=gout[:])

            # ---- P2: sequential LSTM steps ----
            with tc.tile_pool(name="state", bufs=1) as state, \
                 tc.tile_pool(name="p2", bufs=3) as p2, \
                 tc.tile_pool(name="psg", bufs=1, space="PSUM") as psg, \
                 tc.tile_pool(name="pst", bufs=2, space="PSUM") as pst:
                hT_sb = state.tile([128, KH * B_local], F32R)
                c_st = state.tile([B_local, H], F32)
                hring = state.tile([B_local, 2, H], F32)
                nc.vector.memset(hT_sb[:].bitcast(F32), 0.0)
                nc.vector.memset(c_st[:], 0.0)
                for t in range(S):
                    gin = p2.tile([B_local, G], F32, tag="gin")
                    nc.sync.dma_start(
                        out=gin[:], in_=ginp[B_local * t:B_local * (t + 1), :])
                    gps = psg.tile([B_local, G], F32)
                    for n in range(NT):
                        for k in range(KH):
                            nc.tensor.matmul(
                                gps[:, 512 * n:512 * (n + 1)],
                                hT_sb[:, B_local * k:B_local * (k + 1)],
                                whh_sb[:, k, 512 * n:512 * (n + 1)],
                                start=(k == 0), stop=(k == KH - 1))
                    gsb = p2.tile([B_local, G], F32, tag="gsb")
                    for n in range(NT):
                        sl = slice(512 * n, 512 * (n + 1))
                        nc.vector.tensor_add(gsb[:, sl], gps[:, sl],
                                             gin[:, sl])
                    # gate column layout: [i | f | o | g]
                    act = p2.tile([B_local, G], F32, tag="act")
                    nc.scalar.activation(act[:, 0:3 * H], gsb[:, 0:3 * H],
                                         AF.Sigmoid)
                    nc.scalar.activation(act[:, 3 * H:G], gsb[:, 3 * H:G],
                                         AF.Tanh)
                    tmp = p2.tile([B_local, H], F32, tag="tmp")
                    nc.vector.tensor_mul(tmp[:], act[:, 0:H], act[:, 3 * H:G])
                    nc.vector.tensor_mul(c_st[:], c_st[:], act[:, H:2 * H])
                    nc.vector.tensor_add(c_st[:], c_st[:], tmp[:])
                    tnc = p2.tile([B_local, H], F32, tag="tnc")
                    nc.scalar.activation(tnc[:], c_st[:], AF.Tanh)
                    hnew = hring[:, t % 2, :]
                    nc.vector.tensor_mul(hnew, tnc[:], act[:, 2 * H:3 * H])
                    tp = pst.tile([128, KH * B_local], F32)
                    for k in range(KH):
                        nc.tensor.transpose(
                            tp[:, B_local * k:B_local * (k + 1)],
                            hnew[:, 128 * k:128 * (k + 1)],
                            eye_sb[0:B_local, 0:B_local])
                    nc.vector.tensor_copy(hT_sb[:], tp[:])
                    nc.gpsimd.indirect_dma_start(
                        out=hstore[:],
                        out_offset=bass.IndirectOffsetOnAxis(
                            ap=offs_sb[:, t:t + 1], axis=0),
                        in_=hnew, in_offset=None)

            # ---- P3: hstore -> hTstore transposes + preA matmuls ----
            with tc.tile_pool(name="pa", bufs=1) as pa, \
                 tc.tile_pool(name="p3", bufs=3) as p3, \
                 tc.tile_pool(name="ps3", bufs=4, space="PSUM") as ps3, \
                 tc.tile_pool(name="psa", bufs=2, space="PSUM") as psa:
                attw_sb = pa.tile([128, KH, A], F32R)
                nc.sync.dma_start(
                    out=attw_sb[:],
                    in_=attWT[:].rearrange("(k p) a -> p k a", p=128))
                attv_sb = pa.tile([A, 1], F32R)
                nc.sync.dma_start(out=attv_sb[:], in_=attv[:])
                preA = pa.tile([A, SB], F32)
                for j in range(MT):
                    hsl = p3.tile([128, H], F32, tag="hsl")
                    nc.sync.dma_start(
                        out=hsl[:], in_=hstore[128 * j:128 * (j + 1), :])
                    hTt = p3.tile([128, KH, 128], F32R, tag="hTt")
                    for k in range(KH):
                        pt = ps3.tile([128, 128], F32)
                        nc.tensor.transpose(
                            pt[:], hsl[:, 128 * k:128 * (k + 1)], eye_sb[:])
                        nc.vector.tensor_copy(hTt[:, k, :], pt[:])
                    nc.sync.dma_start(
                        out=hTstore[:, 128 * j:128 * (j + 1)].rearrange(
                            "(k p) m -> p k m", p=128),
                        in_=hTt[:])
                    pp = psa.tile([A, 128], F32)
                    for k in range(KH):
                        nc.tensor.matmul(
                            pp[:], attw_sb[:, k, :], hTt[:, k, :],
                            start=(k == 0), stop=(k == KH - 1))
                    nc.vector.tensor_copy(preA[:, 128 * j:128 * (j + 1)],
                                          pp[:])

                # ---- P4: pairwise AllReduce of preA ----
                with tc.tile_pool(name="dram", bufs=1, space="DRAM") as dp:
                    cc_in = dp.tile([A, SB], F32)
                    cc_out = dp.tile([A, SB], F32)
                    nc.sync.dma_start(out=cc_in[:], in_=preA[:])
                    nc.gpsimd.collective_compute(
                        "AllReduce", ALU.add, replica_groups=pairs,
                        ins=[cc_in.opt()], outs=[cc_out.opt()])
                    nc.sync.dma_start(out=preA[:], in_=cc_out[:])

                # ---- P5: tanh + v-dot -> scores[b, s] ----
                tanhT = pa.tile([A, SB], F32R)
                for q in range(4):
                    sl = slice(SB // 4 * q, SB // 4 * (q + 1))
                    nc.scalar.activation(tanhT[:, sl], preA[:, sl], AF.Tanh)
                scoresB = pa.tile([B_local, S], F32)
                tv = tanhT[:].rearrange("p (s b) -> p b s", b=B_local)
                with tc.tile_pool(name="p5", bufs=2) as p5, \
                     tc.tile_pool(name="ps5", bufs=2, space="PSUM") as ps5:
                    for b in range(B_local):
                        pv = ps5.tile([1, S], F32)
                        nc.tensor.matmul(pv[:], attv_sb[:], tv[:, b, :],
                                         start=True, stop=True)
                        sv = p5.tile([1, S], F32)
                        nc.vector.tensor_copy(sv[:], pv[:])
                        nc.sync.dma_start(out=scoresB[b:b + 1, :], in_=sv[:])

                # ---- P6: softmax over s ----
                mx = pa.tile([B_local, 1], F32)
                nc.vector.tensor_reduce(mx[:], scoresB[:],
                                        axis=mybir.AxisListType.X, op=ALU.max)
                nmx = pa.tile([B_local, 1], F32)
                nc.vector.tensor_scalar_mul(nmx[:], mx[:], -1.0)
                ssum = pa.tile([B_local, 1], F32)
                wgt = pa.tile([B_local, S], F32)
                nc.scalar.activation(wgt[:], scoresB[:], AF.Exp,
                                     bias=nmx[:, 0:1], accum_out=ssum[:])
                rs = pa.tile([B_local, 1], F32)
                nc.vector.reciprocal(rs[:], ssum[:])
                nc.vector.tensor_scalar_mul(wgt[:], wgt[:], rs[:, 0:1])

                # ---- P7: pooledT[h, b] = sum_s hT * w ----
                wdram = nc.dram_tensor("wdram", [B_local, S], F32)
                nc.sync.dma_start(out=wdram[:], in_=wgt[:])
                wrep = pa.tile([128, B_local, S], F32)
                nc.gpsimd.dma_start(
                    out=wrep[:], in_=wdram[:].partition_broadcast(128))
                pool_sb = pa.tile([128, KH, B_local], F32)
                with tc.tile_pool(name="p7", bufs=1) as p7:
                    for k in range(KH):
                        hck = p7.tile([128, SB], F32, tag="hck")
                        nc.sync.dma_start(
                            out=hck[:],
                            in_=hTstore[128 * k:128 * (k + 1), :].bitcast(F32))
                        scr = p7.tile([128, S], F32, tag="scr")
                        hv = hck[:].rearrange("p (s b) -> p b s", b=B_local)
                        for b in range(B_local):
                            nc.vector.tensor_mul(scr[:], hv[:, b, :],
                                                 wrep[:, b, :])
                            nc.vector.tensor_reduce(
                                pool_sb[:, k, b:b + 1], scr[:],
                                axis=mybir.AxisListType.X, op=ALU.add)
                nc.sync.dma_start(
                    out=pooledT[:].rearrange("(k p) b -> p k b", p=128),
                    in_=pool_sb[:])

    nc.compile()
    return nc


_PERM = np.r_[0:H, H:2 * H, 3 * H:4 * H, 2 * H:3 * H]  # -> [i, f, o, g]


def _prep_core_inputs(x_shard, Wih, Whh, bih, bhh, att_W, att_v, is_backward):
    x = x_shard[:, ::-1, :] if is_backward else x_shard
    xT = np.ascontiguousarray(x.transpose(2, 1, 0).reshape(D, S * B_LOCAL))
    wihT = np.ascontiguousarray(Wih[_PERM].T)
    whhT = np.ascontiguousarray(Whh[_PERM].T)
    b = np.ascontiguousarray((bih + bhh)[_PERM])[None, :]
    half = att_W[:, H:] if is_backward else att_W[:, :H]
    attWT = np.ascontiguousarray(half.T)
    attv = np.ascontiguousarray(att_v.reshape(A, 1))
    eye = np.eye(128, dtype=np.float32)
    pos = np.arange(S - 1, -1, -1) if is_backward else np.arange(S)
    offsT = (pos[None, :] * B_LOCAL
             + np.arange(B_LOCAL)[:, None]).astype(np.int32)
    return {"xT": xT, "wihT": wihT, "whhT": whhT, "bias": b, "attWT": attWT,
            "attv": attv, "eye": eye, "offsT": offsT}


def make_in_maps(x, f_Wih, f_Whh, f_bih, f_bhh, b_Wih, b_Whh, b_bih, b_bhh,
                 att_W, att_v):
    n_half = N_CORES // 2
    in_maps = []
    for c in range(N_CORES):
        is_b = c >= n_half
        sh = c % n_half
        xs = x[B_LOCAL * sh:B_LOCAL * (sh + 1)]
        if is_b:
            in_maps.append(_prep_core_inputs(
                xs, b_Wih, b_Whh, b_bih, b_bhh, att_W, att_v, True))
        else:
            in_maps.append(_prep_core_inputs(
                xs, f_Wih, f_Whh, f_bih, f_bhh, att_W, att_v, False))
    return in_maps


def assemble_output(results):
    n_half = N_CORES // 2
    out = np.empty((B, 2 * H), np.float32)
    for c in range(N_CORES):
        is_b = c >= n_half
        sh = c % n_half
        cols = slice(H, 2 * H) if is_b else slice(0, H)
        out[B_LOCAL * sh:B_LOCAL * (sh + 1), cols] = results[c]["pooledT"].T
    return out


_PROGRAM = None


def _get_program():
    global _PROGRAM
    if _PROGRAM is None:
        _PROGRAM = build_program()
    return _PROGRAM


def kernel(**inputs):
    nc = _get_program()
    in_maps = make_in_maps(
        np.asarray(inputs["x"], np.float32),
        np.asarray(inputs["f_Wih"], np.float32),
        np.asarray(inputs["f_Whh"], np.float32),
        np.asarray(inputs["f_bih"], np.float32),
        np.asarray(inputs["f_bhh"], np.float32),
        np.asarray(inputs["b_Wih"], np.float32),
        np.asarray(inputs["b_Whh"], np.float32),
        np.asarray(inputs["b_bih"], np.float32),
        np.asarray(inputs["b_bhh"], np.float32),
        np.asarray(inputs["att_W"], np.float32),
        np.asarray(inputs["att_v"], np.float32))
    res = run_bass_kernel_spmd(nc, in_maps, core_ids=list(range(N_CORES)))
    return assemble_output(res.results)
